# revision 1
# baseline (speedup 1.0000x reference)
"""CompressedLinear Trainium2 kernel.

Computes y = x @ (w_int8 * 0.01)^T + bias for
  x      [4, 32, 4096]  fp32
  w_int8 [11008, 4096]  int32 (int8 values)
  bias   [11008]        fp32
  y      [4, 32, 11008] fp32

Strategy (tensor-parallel over output rows, 8 NeuronCores):
- Host: transpose w to wT [4096, 11008] and shard the row dim into
  8 contiguous [4096, 1376] int32 shards so each core streams its shard
  with full-bandwidth, fully-contiguous DMAs. The int32 payload is
  preserved end-to-end: every core reads its full 22.5 MB from HBM; the
  int32->bf16 conversion happens inside the SDMA engines (SWDGE
  cast-DMA), which is exact for int8-valued data (|v| <= 128 is exactly
  representable in bf16) and costs no compute-engine cycles.
- Host: fold the 0.01 dequant scale into x, split x*0.01 into an exact
  bf16 (hi, lo) pair, transpose to [c, t] and swizzle into the SBUF
  layout the PE stationary operand consumes. Two bf16 matmul passes
  (hi + lo) recover fp32-level accuracy because bf16 x bf16 products
  accumulate exactly in fp32 PSUM (rel err ~2e-6 measured).
- Device, per core: stream the weight shard in a few large slabs
  (descending sizes, so the compute tail after the last DMA is tiny).
  For each 128-wide chunk of the contraction dim, load x^T[c] (hi, lo)
  as PE stationary and stream the bf16 weights; accumulate
  y[tokens, rows] in 3 PSUM banks (512/512/352 output rows). The bias
  enters PSUM via K=1 matmuls against a ones-vector (bias split into a
  bf16 hi/lo pair, exact to 2^-17). Per-bank PSUM drains alternate
  ScalarE/VectorE and each bank's output DMA starts as soon as its
  drain lands.
- Host: concatenate the 8 row-shards, reshape.
"""

from contextlib import ExitStack

import numpy as np
import ml_dtypes

ROWS, COLS = 11008, 4096
SCALE = 0.01
T = 128                      # tokens = 4*32
NCORES = 8
RPC = ROWS // NCORES         # 1376 rows per core
CCHUNK = 128                 # contraction tile (partition dim)
NCHUNKS = COLS // CCHUNK     # 32
SLAB_C = 4                   # uniform c-chunks per slab (bench variants)
# slab schedule: uniform 2.8MB DMAs for bandwidth, small final slabs so
# the compute tail after the last weight DMA is short
SLAB_SCHED = [4, 4, 4, 4, 4, 4, 4, 2, 2]
RBLOCKS = [(0, 512), (512, 512), (1024, 352)]

BF16 = ml_dtypes.bfloat16

_cached = {}


def _build_program(reps=1, loop_reps=0, mode="full", cast_dma=True,
                   slab_c=None, wbufs=3, dual=False, sched=None,
                   alt=False):
    """Build the device program. reps>1 repeats the streaming body
    (unrolled); loop_reps>0 wraps the body in a device-side For_i loop.
    mode: "full" | "dma_only" | "pe_only" (benchmark variants).
    cast_dma: SWDGE int32->bf16 cast in DMA; else HWDGE + DVE convert.
    slab_c: uniform slab size override; None uses SLAB_SCHED.
    dual: split each slab DMA across gpsimd/sync/scalar queues."""
    import concourse.mybir as mybir
    import concourse.tile as tile
    from concourse import bacc

    if sched is not None:
        sched = list(sched)
    elif slab_c is None:
        sched = list(SLAB_SCHED)
    else:
        sched = [slab_c] * (NCHUNKS // slab_c)
    assert sum(sched) == NCHUNKS
    max_slab = max(sched)

    nc = bacc.Bacc("TRN2", target_bir_lowering=False, debug=False,
                   enable_asserts=False, num_devices=NCORES)

    # weight shard, host-swizzled to the SBUF slab layout:
    # wT[p, k*RPC + r] = w^T[k*128 + p, r]  ->  every DMA is an identity
    # copy whose per-partition DRAM runs are slab_c*5504B contiguous
    wT = nc.dram_tensor("wT", [CCHUNK, NCHUNKS * RPC], mybir.dt.int32,
                        kind="ExternalInput").ap()
    xhi = nc.dram_tensor("xhi", [CCHUNK, NCHUNKS * T], mybir.dt.bfloat16,
                         kind="ExternalInput").ap()
    xlo = nc.dram_tensor("xlo", [CCHUNK, NCHUNKS * T], mybir.dt.bfloat16,
                         kind="ExternalInput").ap()
    bhi = nc.dram_tensor("bhi", [1, RPC], mybir.dt.bfloat16,
                         kind="ExternalInput").ap()
    blo = nc.dram_tensor("blo", [1, RPC], mybir.dt.bfloat16,
                         kind="ExternalInput").ap()
    out = nc.dram_tensor("out", [T, RPC], mybir.dt.float32,
                         kind="ExternalOutput").ap()

    with tile.TileContext(nc) as tc, ExitStack() as ctx:
        const = ctx.enter_context(tc.tile_pool(name="const", bufs=1))
        wpool = ctx.enter_context(tc.tile_pool(name="w", bufs=wbufs))
        psum = ctx.enter_context(tc.tile_pool(name="psum", bufs=3,
                                              space="PSUM"))
        opool = ctx.enter_context(tc.tile_pool(name="o", bufs=2))

        # x stationaries: [c_part, chunk*token], swizzled on host
        xhi_sb = const.tile([CCHUNK, COLS], mybir.dt.bfloat16, tag="xhi")
        xlo_sb = const.tile([CCHUNK, COLS], mybir.dt.bfloat16, tag="xlo")
        nc.sync.dma_start(out=xhi_sb[:], in_=xhi[:])
        nc.sync.dma_start(out=xlo_sb[:], in_=xlo[:])

        bhi_sb = const.tile([1, RPC], mybir.dt.bfloat16, tag="bhi")
        blo_sb = const.tile([1, RPC], mybir.dt.bfloat16, tag="blo")
        nc.sync.dma_start(out=bhi_sb[:], in_=bhi[:])
        nc.sync.dma_start(out=blo_sb[:], in_=blo[:])

        ones = const.tile([1, T], mybir.dt.bfloat16, tag="ones")
        nc.vector.memset(ones[:], 1.0)

        def body():
            ps = []
            if mode != "dma_only":
                # Seed each PSUM r-block with the bias (broadcast over
                # tokens by a K=1 matmul: ones^T [T] x bias [rn]).
                for r0, rn in RBLOCKS:
                    pt = psum.tile([T, rn], mybir.dt.float32, tag="acc")
                    ps.append(pt)
                    nc.tensor.matmul(pt[:], lhsT=ones[:],
                                     rhs=bhi_sb[:, r0:r0 + rn],
                                     start=True, stop=False)
                    nc.tensor.matmul(pt[:], lhsT=ones[:],
                                     rhs=blo_sb[:, r0:r0 + rn],
                                     start=False, stop=False)

            c0 = 0
            first_slab = None
            for s, sc in enumerate(sched):
                if mode == "pe_only" and s > 0:
                    wslab = first_slab
                    cur = sc
                else:
                    if dual:
                        # split the slab stream across the three DMA issue
                        # paths: SWDGE(cast) + 2x HWDGE(raw int32)
                        wslab = wpool.tile([CCHUNK, max_slab, RPC],
                                           mybir.dt.bfloat16, tag="wslab")
                        wraw = wpool.tile([CCHUNK, max_slab, RPC],
                                          mybir.dt.int32, tag="wraw")
                        src = wT[:, c0 * RPC:(c0 + sc) * RPC].rearrange(
                            "p (j r) -> p j r", r=RPC)
                        h = RPC // 2
                        q = h + RPC // 4
                        nc.gpsimd.dma_start(out=wslab[:, :sc, :h],
                                            in_=src[:, :, :h])
                        nc.sync.dma_start(out=wraw[:, :sc, h:q],
                                          in_=src[:, :, h:q])
                        nc.scalar.dma_start(out=wraw[:, :sc, q:],
                                            in_=src[:, :, q:])
                    elif alt and s % 2 == 1:
                        # odd slabs ride the HWDGE queue (raw int32) with a
                        # DVE convert, overlapping SWDGE issue overheads
                        wraw = wpool.tile([CCHUNK, max_slab, RPC],
                                          mybir.dt.int32, tag="wraw",
                                          bufs=2)
                        nc.sync.dma_start(
                            out=wraw[:, :sc, :],
                            in_=wT[:, c0 * RPC:(c0 + sc) * RPC])
                        wslab = wpool.tile([CCHUNK, max_slab, RPC],
                                           mybir.dt.bfloat16, tag="wslab")
                        nc.vector.tensor_copy(out=wslab[:, :sc, :],
                                              in_=wraw[:, :sc, :])
                    elif cast_dma:
                        if sc <= 2:
                            # tail slabs get dedicated slots so their DMAs
                            # never wait on big-slab buffer release (keeps
                            # the DMA queue streaming through the tail)
                            wslab = wpool.tile([CCHUNK, sc, RPC],
                                               mybir.dt.bfloat16,
                                               tag="wtail", bufs=2)
                        else:
                            wslab = wpool.tile([CCHUNK, max_slab, RPC],
                                               mybir.dt.bfloat16,
                                               tag="wslab")
                        # SWDGE cast-DMA: int32 DRAM -> bf16 SBUF
                        nc.gpsimd.dma_start(
                            out=wslab[:, :sc, :],
                            in_=wT[:, c0 * RPC:(c0 + sc) * RPC])
                    else:
                        wraw = wpool.tile([CCHUNK, max_slab, RPC],
                                          mybir.dt.int32, tag="wraw")
                        nc.sync.dma_start(
                            out=wraw[:, :sc, :],
                            in_=wT[:, c0 * RPC:(c0 + sc) * RPC])
                        wslab = wpool.tile([CCHUNK, max_slab, RPC],
                                           mybir.dt.bfloat16, tag="wslab")
                        nc.vector.tensor_copy(out=wslab[:, :sc, :],
                                              in_=wraw[:, :sc, :])
                    if mode == "pe_only" and s == 0:
                        first_slab = wslab
                if mode == "dma_only":
                    c0 += sc
                    continue
                for j in range(sc):
                    k = c0 + j
                    last_k = k == NCHUNKS - 1
                    if not last_k:
                        for x_sb, is_lo in ((xhi_sb, False), (xlo_sb, True)):
                            lhsT = x_sb[:, k * T:(k + 1) * T]
                            for rb, (r0, rn) in enumerate(RBLOCKS):
                                nc.tensor.matmul(
                                    ps[rb][:], lhsT=lhsT,
                                    rhs=wslab[:, j, r0:r0 + rn],
                                    start=False, stop=False)
                    else:
                        # final chunk: r-block-major so each PSUM bank hits
                        # its stop (and can drain) as early as possible
                        for rb, (r0, rn) in enumerate(RBLOCKS):
                            for x_sb, is_lo in ((xhi_sb, False),
                                                (xlo_sb, True)):
                                lhsT = x_sb[:, k * T:(k + 1) * T]
                                nc.tensor.matmul(
                                    ps[rb][:], lhsT=lhsT,
                                    rhs=wslab[:, j, r0:r0 + rn],
                                    start=False, stop=is_lo)
                c0 += sc

            if mode == "dma_only":
                return
            o_sb = opool.tile([T, RPC], mybir.dt.float32, tag="osb")
            drain = [nc.scalar.copy, nc.vector.tensor_copy, nc.scalar.copy]
            for rb, (r0, rn) in enumerate(RBLOCKS):
                drain[rb](out=o_sb[:, r0:r0 + rn], in_=ps[rb][:])
                nc.sync.dma_start(out=out[:, r0:r0 + rn],
                                  in_=o_sb[:, r0:r0 + rn])

        if loop_reps:
            with tc.For_i(0, loop_reps, 1):
                body()
        else:
            for _rep in range(reps):
                body()

    nc.compile()
    return nc


def _get_program():
    if "nc" not in _cached:
        _cached["nc"] = _build_program()
    return _cached["nc"]


def _prep_inputs(x, w_int8, bias):
    xs = (x.reshape(T, COLS).astype(np.float32) * np.float32(SCALE))
    xhi = xs.astype(BF16)
    xlo = (xs - xhi.astype(np.float32)).astype(BF16)

    def swizzle(a):
        # [T, COLS] -> [p, k*T + t] = x^T[k*128+p, t]: the exact SBUF
        # layout the PE stationary slices consume.
        return np.ascontiguousarray(
            a.reshape(T, NCHUNKS, CCHUNK).transpose(2, 1, 0)
        ).reshape(CCHUNK, NCHUNKS * T)

    xhi_dev = swizzle(xhi)
    xlo_dev = swizzle(xlo)

    bh = bias.astype(BF16)
    bl = (bias.astype(np.float32) - bh.astype(np.float32)).astype(BF16)
    bh_sh = np.ascontiguousarray(bh.reshape(NCORES, 1, RPC))
    bl_sh = np.ascontiguousarray(bl.reshape(NCORES, 1, RPC))

    # wT shards in SBUF slab layout: [core, p, k*RPC + r] = w[s*RPC + r,
    # k*128 + p] so device DMAs are identity copies with 22KB-contiguous
    # per-partition runs.
    w4 = w_int8.reshape(NCORES, RPC, NCHUNKS, CCHUNK)
    wT_sh = np.ascontiguousarray(w4.transpose(0, 3, 2, 1)).reshape(
        NCORES, CCHUNK, NCHUNKS * RPC)
    return xhi_dev, xlo_dev, bh_sh, bl_sh, wT_sh


def kernel(x, w_int8, bias):
    from concourse import bass_utils

    nc = _get_program()
    xhi_dev, xlo_dev, bh_sh, bl_sh, wT_sh = _prep_inputs(
        np.asarray(x), np.asarray(w_int8), np.asarray(bias))

    in_maps = [
        {"wT": wT_sh[s], "xhi": xhi_dev, "xlo": xlo_dev,
         "bhi": bh_sh[s], "blo": bl_sh[s]}
        for s in range(NCORES)
    ]
    res = bass_utils.run_bass_kernel_spmd(nc, in_maps,
                                          core_ids=list(range(NCORES)))
    shards = [res.results[s]["out"] for s in range(NCORES)]
    y = np.concatenate(shards, axis=1).reshape(4, 32, ROWS)
    return np.ascontiguousarray(y.astype(np.float32))

